# revision 31
# baseline (speedup 1.0000x reference)
"""KMeans assignment kernel (retrieval_knn) for 8 Trainium2 NeuronCores.

Computes argmin_k ||x_n - c_k||^2 for x [262144, 64] f32 against centers
[1024, 64] f32, returning int32 cluster ids [262144].

argmin ||x-c||^2 == argmax s, s = 2x.c - ||c||^2.  Centers are pre-combined
on the host into PAIR sums/differences, so the PE emits, per point, the 512
values sum'_g = (s_2g + s_2g+1)/2 and diff'_g = (s_2g - s_2g+1)/2 (fp16
single-pass matmuls, 2x512 columns per tile).  The idle ACT engine computes
|diff'| (one Abs activation per tile), and ONE 2-stream custom DVE op per
tile (ARGMAXS_ANT: argmax over Src0+Src1 = sum'+|diff'| = max(s_2g, s_2g+1))
returns the winning PAIR index g* from a 512-wide scan — HALF the scan of a
1024-wide argmax.  The within-pair winner (1 bit) is resolved exactly on the
host by comparing the two candidate centers per point in fp64 (O(N) numpy).
No spill, no gather, no reduce cascades; DVE does one 512-elem op per tile.
"""

import numpy as np

N_POINTS = 262144
N_FEATURES = 64
N_CLUSTERS = 1024
N_PAIRS = N_CLUSTERS // 2               # 512
N_CORES = 8
PTS_PER_CORE = N_POINTS // N_CORES      # 32768
TILE_P = 128                            # points per tile (partition dim)
N_TILES = PTS_PER_CORE // TILE_P        # 256
BT = 32                                 # tiles per output batch

_CACHE = {}


def _register_ops():
    """Register the custom DVE ops (runtime append to dve_ops.OPS).

    ARGMAX_ANT  (row 17): accum = argmax_k Src0[k]            (kept for row
                          stability; unused by this kernel)
    ARGMAXS_ANT (row 18): accum = argmax_k (Src0[k] + Src1[k]) (last tie)
    """
    from concourse import dve_ops
    from concourse.dve_spec import (
        Spec, Src0, Src1, Idx, MaxNeg, AluOp, scan, eq, select, maxx,
    )

    if "ARGMAXS_ANT" in dve_ops._SUB_OPCODE_FOR_NAME:
        # already registered (e.g. module re-import in the same process):
        # recover the op object from the registry, not module state
        op2 = next(op for op in dve_ops.OPS if op.name == "ARGMAXS_ANT")
        _CACHE["argmaxs_op"] = op2
        return op2

    def _ref_argmax(in0, in1, s0, s1, imm2):
        r = np.maximum.accumulate(in0, axis=-1)
        idx = np.arange(in0.shape[-1], dtype=np.float32)
        return np.where(in0 == r, idx, -np.finfo(np.float32).max)

    def _ref_argmaxs(in0, in1, s0, s1, imm2):
        m = in0.astype(np.float32) + in1
        r = np.maximum.accumulate(m, axis=-1)
        idx = np.arange(m.shape[-1], dtype=np.float32)
        return np.where(m == r, idx, -np.finfo(np.float32).max)

    op1 = dve_ops.DveOp(
        "ARGMAX_ANT",
        Spec(
            body=select(eq(Src0, scan(AluOp.MAX, Src0)), Idx, MaxNeg),
            accum=maxx,
            reference=_ref_argmax,
        ),
        subdim=False,
        uops_sha={"v3": "d14dbf28477fed0e", "v4": "7311a447fa794d46"},
    )
    _mp = Src0 + Src1
    op2 = dve_ops.DveOp(
        "ARGMAXS_ANT",
        Spec(
            body=select(eq(_mp, scan(AluOp.MAX, _mp)), Idx, MaxNeg),
            accum=maxx,
            reference=_ref_argmaxs,
        ),
        subdim=False,
        uops_sha={"v3": "86f16b92aa28dba0", "v4": "0dff67e8a1d91028"},
    )
    for op in (op1, op2):
        dve_ops.OPS.append(op)
        dve_ops._SUB_OPCODE_FOR_NAME[op.name] = (
            dve_ops._CUSTOM_DVE_ROW_BASE + len(dve_ops.OPS) - 1
        )
        dve_ops.CUSTOM_DVE_SPECS[op.name] = op.spec
    _CACHE["argmaxs_op"] = op2
    return op2


def _build_bass():
    import concourse.bass as bass
    import concourse.bacc as bacc
    import concourse.mybir as mybir
    import concourse.tile as tile
    from contextlib import ExitStack

    argmaxs_op = _register_ops()

    f16 = mybir.dt.float16
    f32 = mybir.dt.float32
    u32 = mybir.dt.uint32

    nc = bacc.Bacc(None, target_bir_lowering=False)

    xq = nc.declare_dram_parameter("xq", [67, PTS_PER_CORE], f16, isOutput=False)
    ccs = nc.declare_dram_parameter("ccs", [67, N_PAIRS], f16, isOutput=False)
    ccd = nc.declare_dram_parameter("ccd", [67, N_PAIRS], f16, isOutput=False)
    out = nc.declare_dram_parameter("out", [128, N_TILES], u32, isOutput=True)

    with tile.TileContext(nc) as tc, ExitStack() as ctx:
        const_pool = ctx.enter_context(tc.tile_pool(name="const", bufs=1))
        psum_pool = ctx.enter_context(
            tc.tile_pool(name="psum", bufs=4, space=bass.MemorySpace.PSUM)
        )
        abs_pool = ctx.enter_context(tc.tile_pool(name="absd", bufs=4))
        scr_pool = ctx.enter_context(tc.tile_pool(name="scr", bufs=3))
        idx_pool = ctx.enter_context(tc.tile_pool(name="idx", bufs=3))
        out_pool = ctx.enter_context(tc.tile_pool(name="out", bufs=1))

        ccs_t = const_pool.tile([67, N_PAIRS], f16)
        nc.gpsimd.dma_start(ccs_t[:], ccs[:])
        ccd_t = const_pool.tile([67, N_PAIRS], f16)
        nc.gpsimd.dma_start(ccd_t[:], ccd[:])
        xq_t = const_pool.tile([67, PTS_PER_CORE], f16)
        XCH = 32
        CHW = PTS_PER_CORE // XCH
        for ch in range(XCH):
            csl = slice(ch * CHW, (ch + 1) * CHW)
            nc.sync.dma_start(xq_t[:, csl], xq[:, csl])

        # warm the PE p-state during the x-load dead time
        ps2 = psum_pool.tile([128, 2, N_PAIRS], f32)
        for _ in range(8):
            nc.tensor.matmul(
                ps2[:, 0, :], ccs_t[:, 0:TILE_P], ccs_t[:],
                start=True, stop=True,
            )

        outbuf = out_pool.tile([128, N_TILES], u32)

        for t in range(N_TILES):
            i = t % BT
            if i == 0:
                idxb = idx_pool.tile([128, BT], f32)
            tsl = slice(t * TILE_P, (t + 1) * TILE_P)
            ps2 = psum_pool.tile([128, 2, N_PAIRS], f32)
            nc.tensor.matmul(
                ps2[:, 0, :], xq_t[:, tsl], ccs_t[:], start=True, stop=True
            )
            nc.tensor.matmul(
                ps2[:, 1, :], xq_t[:, tsl], ccd_t[:], start=True, stop=True
            )
            absd = abs_pool.tile([128, N_PAIRS], f32)
            nc.scalar.activation(
                absd[:], ps2[:, 1, :], mybir.ActivationFunctionType.Abs
            )
            scratch = scr_pool.tile([128, N_PAIRS], f32)
            nc.vector._custom_dve(
                argmaxs_op,
                out=scratch[:],
                in0=ps2[:, 0, :],
                in1=absd[:],
                accum_out=idxb[:, i : i + 1],
            )
            if i == BT - 1:
                tb = t // BT
                nc.vector.tensor_copy(outbuf[:, tb * BT : (tb + 1) * BT], idxb[:])

        nc.sync.dma_start(out[:], outbuf[:])

    nc.compile()
    return nc


def _casc3(A):
    """3-row fp16 cascade summing (exactly, up to fp16 subnormal flush) to A."""
    f16 = np.float16
    n1 = A.astype(f16)
    r1 = A - n1.astype(np.float64)
    n2 = r1.astype(f16)
    n3 = (r1 - n2.astype(np.float64)).astype(f16)
    return n1, n2, n3


def _prep(x: np.ndarray, centers: np.ndarray):
    f16 = np.float16
    xd = x.astype(np.float64)
    cd = centers.astype(np.float64)

    xq = np.empty((67, N_POINTS), f16)
    xq[0:64] = np.ascontiguousarray(xd.T).astype(f16)
    xq[64:67] = f16(1.0)

    cn = (cd * cd).sum(1)
    csum = cd[0::2] + cd[1::2]                  # [512, 64]
    cdif = cd[0::2] - cd[1::2]
    cnsum = (cn[0::2] + cn[1::2]) / 2.0
    cndif = (cn[0::2] - cn[1::2]) / 2.0

    # device computes sum'_g = x.csum - cnsum = (s_2g + s_2g+1)/2
    #             and diff'_g = x.cdif - cndif = (s_2g - s_2g+1)/2
    ccs = np.empty((67, N_PAIRS), f16)
    ccs[0:64] = csum.T.astype(f16)
    ccs[64], ccs[65], ccs[66] = _casc3(-cnsum)
    ccd = np.empty((67, N_PAIRS), f16)
    ccd[0:64] = cdif.T.astype(f16)
    ccd[64], ccd[65], ccd[66] = _casc3(-cndif)
    return xq, ccs, ccd


def kernel(x: np.ndarray, centers: np.ndarray) -> np.ndarray:
    import sys
    if "/opt/trn_rl_repo" not in sys.path:
        sys.path.insert(0, "/opt/trn_rl_repo")
    from concourse.bass_utils import run_bass_kernel_spmd

    x = np.asarray(x, dtype=np.float32)
    centers = np.asarray(centers, dtype=np.float32)

    xq, ccs, ccd = _prep(x, centers)

    if "nc" not in _CACHE:
        _CACHE["nc"] = _build_bass()
    nc = _CACHE["nc"]

    in_maps = []
    for c in range(N_CORES):
        sl = slice(c * PTS_PER_CORE, (c + 1) * PTS_PER_CORE)
        in_maps.append(
            {
                "xq": np.ascontiguousarray(xq[:, sl]),
                "ccs": ccs,
                "ccd": ccd,
            }
        )

    res = run_bass_kernel_spmd(nc, in_maps, list(range(N_CORES)))

    outs = []
    for c in range(N_CORES):
        o = res.results[c]["out"]                       # [128, N_TILES] uint32
        outs.append(np.asarray(o).astype(np.int64).T.reshape(-1))  # point t*128+p
    g = np.concatenate(outs)                            # winning pair per point

    # within-pair refinement on host: exact fp64 distance compare of the two
    # candidate centers; ties pick the first (matches reference argmin)
    xd = x.astype(np.float64)
    cd = centers.astype(np.float64)
    c0 = cd[2 * g]
    c1 = cd[2 * g + 1]
    d0 = ((xd - c0) ** 2).sum(1)
    d1 = ((xd - c1) ** 2).sum(1)
    ids = np.where(d1 < d0, 2 * g + 1, 2 * g)
    return ids.astype(np.int32)


if __name__ == "__main__":
    rng = np.random.default_rng(0)
    x = rng.normal(size=(N_POINTS, N_FEATURES)).astype(np.float32)
    c = rng.normal(size=(N_CLUSTERS, N_FEATURES)).astype(np.float32)
    ids = kernel(x=x, centers=c)
    d = (
        np.sum(x * x, 1)[:, None]
        - 2.0 * (x @ c.T)
        + np.sum(c * c, 1)[None, :]
    )
    ref = np.argmin(np.abs(d), axis=1)
    print("mismatch:", np.mean(ids != ref))
